# revision 2
# baseline (speedup 1.0000x reference)
"""Trainium2 Bass kernel: causal depthwise Conv1d (K=4) + SiLU.

Reference computation (B=4, S=4096, D=2048):
    y[b, s, d] = silu( sum_k w[d, 0, k] * x[b, s-3+k, d] )   (zero-padded left)

Strategy:
  * Host: transpose x to channel-major (D, B, S), left-pad each row with
    4 zeros (row length 4100), cast to bf16, shard D across the 8
    NeuronCores (256 channels each).  Depthwise conv is channel-independent
    -> no inter-core communication.
  * Core: 8 tiles of [128, 4100].  A tile is computed either on the
    TensorEngine (diag-stationary matmuls accumulating the 4 taps in PSUM,
    ACT silu drains PSUM -> bf16 SBUF) or on the VectorEngine
    (4 tensor_scalar muls @4x + 2 adds @2x, final add on GpSimd, ACT silu).
    Inputs stream in 2-chunk DMAs on the sync queue (HWDGE), outputs leave
    via gpsimd (SWDGE) so blocked output issues never stall input issue.
  * Host: gather, transpose back, cast to f32.
"""

import os
import sys

sys.path.insert(0, "/opt/trn_rl_repo")

import numpy as np
import ml_dtypes

N_CORES = 8
B, S, D = 4, 4096, 2048
K = 4
PAD = 4
ROW = S + PAD  # 4100
D_LOCAL = D // N_CORES  # 256
G = D_LOCAL // 128  # 2 partition groups per core

MM_N = int(os.environ.get("KERNEL_MM_N", "512"))
N_DVE_TILES = int(os.environ.get("KERNEL_N_DVE", "3"))
GP_ADD = bool(int(os.environ.get("KERNEL_GP_ADD", "1")))
IN_CHUNKS = int(os.environ.get("KERNEL_IN_CHUNKS", "2"))

_CACHE = {}


def _build():
    import concourse.tile as tile
    from concourse import bacc, mybir

    nc = bacc.Bacc("TRN2", debug=False, enable_asserts=False, num_devices=N_CORES)
    bf16 = mybir.dt.bfloat16
    f32 = mybir.dt.float32

    x_ap = nc.dram_tensor("x", [G, 128, B, ROW], bf16, kind="ExternalInput").ap()
    wd_ap = nc.dram_tensor("wd", [128, G * K * 128], bf16, kind="ExternalInput").ap()
    w_ap = nc.dram_tensor("w", [128, G * K], f32, kind="ExternalInput").ap()
    out_ap = nc.dram_tensor("out", [G, 128, B, S], bf16, kind="ExternalOutput").ap()

    dve_tiles = {
        0: set(), 1: {1}, 2: {1, 4}, 3: {1, 3, 6}, 4: {1, 3, 5, 7},
    }[N_DVE_TILES]

    with tile.TileContext(nc) as tc:
        with (
            tc.tile_pool(name="wp", bufs=1) as wp,
            tc.tile_pool(name="xp", bufs=8) as xp,
            tc.tile_pool(name="tp", bufs=2) as tp,
            tc.tile_pool(name="cp", bufs=2) as cp,
            tc.tile_pool(name="ps", bufs=2, space="PSUM") as ps,
            tc.tile_pool(name="yp", bufs=4) as yp,
        ):
            # weights via HWDGE on the scalar queue: fast setup, keeps both
            # the sync (input) and gpsimd (output) queues clean
            wd = wp.tile([128, G * K * 128], bf16, tag="wd")
            wt = wp.tile([128, G * K], f32, tag="wt")
            nc.scalar.dma_start(out=wd[:], in_=wd_ap[:])
            nc.scalar.dma_start(out=wt[:], in_=w_ap[:])

            def wdiag(g, k):
                c0 = (g * K + k) * 128
                return wd[:, c0 : c0 + 128]

            # all input chunk DMAs first, in tile order, on the sync queue
            xts = []
            for ti in range(G * B):
                g, b = divmod(ti, B)
                xt = xp.tile([128, ROW], bf16, tag="xt")
                step = (ROW + IN_CHUNKS - 1) // IN_CHUNKS
                for c0 in range(0, ROW, step):
                    cw = min(step, ROW - c0)
                    nc.sync.dma_start(
                        out=xt[:, c0 : c0 + cw], in_=x_ap[g, :, b, c0 : c0 + cw]
                    )
                xts.append(xt)

            def emit_dve(g, b, xt, lo, hi):
                # y[s] = sum_k w_k * xt[s + 1 + k] on the vector engine
                W = hi - lo

                def wcol(k):
                    return wt[:, g * K + k : g * K + k + 1]

                ts = []
                for k in range(K):
                    t = tp.tile([128, W], bf16, tag=f"t{k % 2}")
                    nc.vector.tensor_scalar_mul(
                        t[:], xt[:, lo + 1 + k : lo + 1 + k + W], wcol(k)
                    )
                    ts.append(t)
                p0 = cp.tile([128, W], bf16, tag="p0")
                nc.vector.tensor_add(p0[:], ts[0][:], ts[1][:])
                p1 = cp.tile([128, W], bf16, tag="p1")
                nc.vector.tensor_add(p1[:], ts[2][:], ts[3][:])
                c = cp.tile([128, W], bf16, tag="c")
                if GP_ADD:
                    nc.gpsimd.tensor_tensor(c[:], p0[:], p1[:], mybir.AluOpType.add)
                else:
                    nc.vector.tensor_add(c[:], p0[:], p1[:])
                y = yp.tile([128, W], bf16, tag="y")
                nc.scalar.activation(
                    out=y[:], in_=c[:], func=mybir.ActivationFunctionType.Silu
                )
                nc.gpsimd.dma_start(out=out_ap[g, :, b, lo:hi], in_=y[:])

            def emit_pe(g, b, xt, lo, hi, last=False):
                cw = hi - lo
                y = yp.tile([128, cw], bf16, tag="y")
                acc = ps.tile([128, cw], f32, tag="acc")
                for k in range(K):
                    for n0 in range(0, cw, MM_N):
                        xlo = lo + n0 + 1 + k
                        nc.tensor.matmul(
                            acc[:, n0 : n0 + MM_N],
                            wdiag(g, k),
                            xt[:, xlo : xlo + MM_N],
                            start=(k == 0),
                            stop=(k == K - 1),
                        )
                if last:
                    # fine silu drain on the final chunk; last out on scalar
                    for s0 in range(0, cw, 1024):
                        nc.scalar.activation(
                            out=y[:, s0 : s0 + 1024],
                            in_=acc[:, s0 : s0 + 1024],
                            func=mybir.ActivationFunctionType.Silu,
                        )
                    nc.scalar.dma_start(out=out_ap[g, :, b, lo:hi], in_=y[:])
                else:
                    nc.scalar.activation(
                        out=y[:], in_=acc[:], func=mybir.ActivationFunctionType.Silu
                    )
                    nc.gpsimd.dma_start(out=out_ap[g, :, b, lo:hi], in_=y[:])

            n_tiles = G * B
            for ti in range(n_tiles):
                g, b = divmod(ti, B)
                xt = xts[ti]
                if ti in dve_tiles:
                    emit_dve(g, b, xt, 0, 2048)
                    emit_dve(g, b, xt, 2048, S)
                else:
                    last_tile = ti == n_tiles - 1
                    emit_pe(g, b, xt, 0, 2048)
                    emit_pe(g, b, xt, 2048, S, last=last_tile)

    nc.compile()
    return nc


def _get_nc():
    if "nc" not in _CACHE:
        _CACHE["nc"] = _build()
    return _CACHE["nc"]


def _make_in_maps(x, w):
    x = np.asarray(x, dtype=np.float32)
    w = np.asarray(w, dtype=np.float32)

    # (B, S, D) -> (D, B, S), bf16, left-pad rows with PAD zeros.
    x_t = np.ascontiguousarray(x.transpose(2, 0, 1)).astype(ml_dtypes.bfloat16)
    x_pad = np.zeros((D, B, ROW), dtype=ml_dtypes.bfloat16)
    x_pad[:, :, PAD:] = x_t
    w_flat = np.ascontiguousarray(w[:, 0, :])  # (D, K) f32

    in_maps = []
    for i in range(N_CORES):
        lo, hi = i * D_LOCAL, (i + 1) * D_LOCAL
        m = {"x": np.ascontiguousarray(x_pad[lo:hi].reshape(G, 128, B, ROW))}
        m["w"] = np.ascontiguousarray(
            w_flat[lo:hi].reshape(G, 128, K).transpose(1, 0, 2).reshape(128, G * K)
        )
        # diag stationaries, laid out [128, G*K*128] partition-first
        wd = np.zeros((G, K, 128, 128), dtype=ml_dtypes.bfloat16)
        wl = w_flat[lo:hi].reshape(G, 128, K).astype(ml_dtypes.bfloat16)
        idx = np.arange(128)
        for g in range(G):
            for k in range(K):
                wd[g, k, idx, idx] = wl[g, :, k]
        # (G,K,p,m) -> (p, G,K,m) -> [128, G*K*128]
        m["wd"] = np.ascontiguousarray(
            wd.transpose(2, 0, 1, 3).reshape(128, G * K * 128)
        )
        in_maps.append(m)
    return in_maps


def _assemble(results):
    parts = []
    for r in results:
        y = np.asarray(r["out"]).reshape(D_LOCAL, B, S)
        parts.append(y)
    y_full = np.concatenate(parts, axis=0)  # (D, B, S) bf16
    return np.ascontiguousarray(y_full.transpose(1, 2, 0)).astype(np.float32)


def kernel(x, w):
    from concourse.bass_utils import run_bass_kernel_spmd

    nc = _get_nc()
    in_maps = _make_in_maps(x, w)
    trace = bool(int(os.environ.get("KERNEL_TRACE", "0")))
    res = None
    err = None
    for attempt in range(3):
        try:
            res = run_bass_kernel_spmd(
                nc, in_maps, core_ids=list(range(N_CORES)),
                trace=trace and attempt == 0,
            )
            break
        except Exception as e:  # transient NRT device errors / missing trace hook
            err = e
            os.environ["BASS_NEVER_TRACE"] = "1"
            trace = False
    if res is None:
        raise err
    _CACHE["last_results"] = res
    return _assemble(res.results)


# revision 5
# speedup vs baseline: 1.1979x; 1.1979x over previous
"""Trainium2 Bass kernel: causal depthwise Conv1d (K=4) + SiLU.

Reference computation (B=4, S=4096, D=2048):
    y[b, s, d] = silu( sum_k w[d, 0, k] * x[b, s-3+k, d] )   (zero-padded left)

Strategy:
  * Host: transpose x to channel-major (D, B, S), left-pad each row with
    4 zeros (row length 4100), cast to bf16, shard D across the 8
    NeuronCores (256 channels each).  Depthwise conv is channel-independent
    -> no inter-core communication.
  * Core: 8 tiles of [128, 4100].  A tile is computed either on the
    TensorEngine (diag-stationary matmuls accumulating the 4 taps in PSUM,
    ACT silu drains PSUM -> bf16 SBUF) or on the VectorEngine
    (4 tensor_scalar muls @4x + 2 adds @2x, final add on GpSimd, ACT silu).
    Inputs stream in 2-chunk DMAs on the sync queue (HWDGE), outputs leave
    via gpsimd (SWDGE) so blocked output issues never stall input issue.
  * Host: gather, transpose back, cast to f32.
"""

import os
import sys

sys.path.insert(0, "/opt/trn_rl_repo")

import numpy as np
import ml_dtypes

N_CORES = 8
B, S, D = 4, 4096, 2048
K = 4
PAD = 4
ROW = S + PAD  # 4100
D_LOCAL = D // N_CORES  # 256
G = D_LOCAL // 128  # 2 partition groups per core

MM_N = int(os.environ.get("KERNEL_MM_N", "512"))
N_DVE_TILES = int(os.environ.get("KERNEL_N_DVE", "3"))
GP_ADD = bool(int(os.environ.get("KERNEL_GP_ADD", "0")))
IN_CHUNKS = int(os.environ.get("KERNEL_IN_CHUNKS", "2"))

_CACHE = {}


def _build():
    import concourse.tile as tile
    from concourse import bacc, mybir

    nc = bacc.Bacc("TRN2", debug=False, enable_asserts=False, num_devices=N_CORES)
    bf16 = mybir.dt.bfloat16
    f32 = mybir.dt.float32

    x_ap = nc.dram_tensor("x", [G, 128, B, ROW], bf16, kind="ExternalInput").ap()
    wd_ap = nc.dram_tensor("wd", [128, G * K * 128], bf16, kind="ExternalInput").ap()
    w_ap = nc.dram_tensor("w", [128, G * K], f32, kind="ExternalInput").ap()
    out_ap = nc.dram_tensor("out", [G, 128, B, S], bf16, kind="ExternalOutput").ap()

    dve_tiles = {
        0: set(), 1: {1}, 2: {1, 4}, 3: {1, 3, 6}, 4: {1, 3, 5, 7},
    }[N_DVE_TILES]

    with tile.TileContext(nc) as tc:
        with (
            tc.tile_pool(name="wp", bufs=1) as wp,
            tc.tile_pool(name="xp", bufs=8) as xp,
            tc.tile_pool(name="tp", bufs=2) as tp,
            tc.tile_pool(name="cp", bufs=2) as cp,
            tc.tile_pool(name="ps", bufs=2, space="PSUM") as ps,
            tc.tile_pool(name="yp", bufs=4) as yp,
        ):
            # weights via HWDGE on the scalar queue: fast setup, keeps both
            # the sync (input) and gpsimd (output) queues clean
            wd = wp.tile([128, G * K * 128], bf16, tag="wd")
            wt = wp.tile([128, G * K], f32, tag="wt")
            nc.scalar.dma_start(out=wd[:], in_=wd_ap[:])
            nc.scalar.dma_start(out=wt[:], in_=w_ap[:])

            def wdiag(g, k):
                c0 = (g * K + k) * 128
                return wd[:, c0 : c0 + 128]

            # all input chunk DMAs first, in tile order, on the sync queue
            xts = []
            for ti in range(G * B):
                g, b = divmod(ti, B)
                xt = xp.tile([128, ROW], bf16, tag="xt")
                step = (ROW + IN_CHUNKS - 1) // IN_CHUNKS
                for c0 in range(0, ROW, step):
                    cw = min(step, ROW - c0)
                    nc.sync.dma_start(
                        out=xt[:, c0 : c0 + cw], in_=x_ap[g, :, b, c0 : c0 + cw]
                    )
                xts.append(xt)

            def emit_dve(g, b, xt, lo, hi):
                # y[s] = sum_k w_k * xt[s + 1 + k] on the vector engine
                W = hi - lo

                def wcol(k):
                    return wt[:, g * K + k : g * K + k + 1]

                ts = []
                for k in range(K):
                    t = tp.tile([128, W], bf16, tag=f"t{k % 2}")
                    nc.vector.tensor_scalar_mul(
                        t[:], xt[:, lo + 1 + k : lo + 1 + k + W], wcol(k)
                    )
                    ts.append(t)
                p0 = cp.tile([128, W], bf16, tag="p0")
                nc.vector.tensor_add(p0[:], ts[0][:], ts[1][:])
                p1 = cp.tile([128, W], bf16, tag="p1")
                nc.vector.tensor_add(p1[:], ts[2][:], ts[3][:])
                c = cp.tile([128, W], bf16, tag="c")
                if GP_ADD:
                    nc.gpsimd.tensor_tensor(c[:], p0[:], p1[:], mybir.AluOpType.add)
                else:
                    nc.vector.tensor_add(c[:], p0[:], p1[:])
                y = yp.tile([128, W], bf16, tag="y")
                nc.scalar.activation(
                    out=y[:], in_=c[:], func=mybir.ActivationFunctionType.Silu
                )
                for c0 in range(0, W, 2048):
                    cw = min(2048, W - c0)
                    nc.gpsimd.dma_start(
                        out=out_ap[g, :, b, lo + c0 : lo + c0 + cw],
                        in_=y[:, c0 : c0 + cw],
                    )

            def emit_pe(g, b, xt, lo, hi, last=False):
                cw = hi - lo
                y = yp.tile([128, cw], bf16, tag="y")
                acc = ps.tile([128, cw], f32, tag="acc")
                for k in range(K):
                    for n0 in range(0, cw, MM_N):
                        xlo = lo + n0 + 1 + k
                        nc.tensor.matmul(
                            acc[:, n0 : n0 + MM_N],
                            wdiag(g, k),
                            xt[:, xlo : xlo + MM_N],
                            start=(k == 0),
                            stop=(k == K - 1),
                        )
                if last:
                    # fine silu drain on the final chunk; last out on scalar
                    for s0 in range(0, cw, 1024):
                        nc.scalar.activation(
                            out=y[:, s0 : s0 + 1024],
                            in_=acc[:, s0 : s0 + 1024],
                            func=mybir.ActivationFunctionType.Silu,
                        )
                    nc.scalar.dma_start(out=out_ap[g, :, b, lo:hi], in_=y[:])
                else:
                    nc.scalar.activation(
                        out=y[:], in_=acc[:], func=mybir.ActivationFunctionType.Silu
                    )
                    nc.gpsimd.dma_start(out=out_ap[g, :, b, lo:hi], in_=y[:])

            n_tiles = G * B
            for ti in range(n_tiles):
                g, b = divmod(ti, B)
                xt = xts[ti]
                if ti in dve_tiles:
                    emit_dve(g, b, xt, 0, S)
                else:
                    last_tile = ti == n_tiles - 1
                    emit_pe(g, b, xt, 0, 2048)
                    emit_pe(g, b, xt, 2048, S, last=last_tile)

    nc.compile()
    return nc


def _get_nc():
    if "nc" not in _CACHE:
        _CACHE["nc"] = _build()
    return _CACHE["nc"]


def _make_in_maps(x, w):
    x = np.asarray(x, dtype=np.float32)
    w = np.asarray(w, dtype=np.float32)

    # (B, S, D) -> (D, B, S), bf16, left-pad rows with PAD zeros.
    x_t = np.ascontiguousarray(x.transpose(2, 0, 1)).astype(ml_dtypes.bfloat16)
    x_pad = np.zeros((D, B, ROW), dtype=ml_dtypes.bfloat16)
    x_pad[:, :, PAD:] = x_t
    w_flat = np.ascontiguousarray(w[:, 0, :])  # (D, K) f32

    in_maps = []
    for i in range(N_CORES):
        lo, hi = i * D_LOCAL, (i + 1) * D_LOCAL
        m = {"x": np.ascontiguousarray(x_pad[lo:hi].reshape(G, 128, B, ROW))}
        m["w"] = np.ascontiguousarray(
            w_flat[lo:hi].reshape(G, 128, K).transpose(1, 0, 2).reshape(128, G * K)
        )
        # diag stationaries, laid out [128, G*K*128] partition-first
        wd = np.zeros((G, K, 128, 128), dtype=ml_dtypes.bfloat16)
        wl = w_flat[lo:hi].reshape(G, 128, K).astype(ml_dtypes.bfloat16)
        idx = np.arange(128)
        for g in range(G):
            for k in range(K):
                wd[g, k, idx, idx] = wl[g, :, k]
        # (G,K,p,m) -> (p, G,K,m) -> [128, G*K*128]
        m["wd"] = np.ascontiguousarray(
            wd.transpose(2, 0, 1, 3).reshape(128, G * K * 128)
        )
        in_maps.append(m)
    return in_maps


def _assemble(results):
    parts = []
    for r in results:
        y = np.asarray(r["out"]).reshape(D_LOCAL, B, S)
        parts.append(y)
    y_full = np.concatenate(parts, axis=0)  # (D, B, S) bf16
    return np.ascontiguousarray(y_full.transpose(1, 2, 0)).astype(np.float32)


def kernel(x, w):
    from concourse.bass_utils import run_bass_kernel_spmd

    nc = _get_nc()
    in_maps = _make_in_maps(x, w)
    trace = bool(int(os.environ.get("KERNEL_TRACE", "0")))
    res = None
    err = None
    for attempt in range(3):
        try:
            res = run_bass_kernel_spmd(
                nc, in_maps, core_ids=list(range(N_CORES)),
                trace=trace and attempt == 0,
            )
            break
        except Exception as e:  # transient NRT device errors / missing trace hook
            err = e
            os.environ["BASS_NEVER_TRACE"] = "1"
            trace = False
    if res is None:
        raise err
    _CACHE["last_results"] = res
    return _assemble(res.results)


# revision 6
# speedup vs baseline: 1.2796x; 1.0682x over previous
"""Trainium2 Bass kernel: causal depthwise Conv1d (K=4) + SiLU.

Reference computation (B=4, S=4096, D=2048):
    y[b, s, d] = silu( sum_k w[d, 0, k] * x[b, s-3+k, d] )   (zero-padded left)

Strategy:
  * Host: transpose x to channel-major (D, B, S), left-pad each row with
    4 zeros (row length 4100), cast to bf16, shard D across the 8
    NeuronCores (256 channels each).  Depthwise conv is channel-independent
    -> no inter-core communication.
  * Core: 8 tiles of [128, 4100].  Tiles are computed on the TensorEngine
    (diag-stationary matmuls accumulate the 4 taps in PSUM; ACT silu drains
    PSUM -> bf16) or the VectorEngine (4 tensor_scalar muls @4x + 3 adds
    @2x; ACT silu).  One tile is split between the two for balance.
    Compute/drain units are emitted in modeled completion order so the
    strict-FIFO ACT queue never head-of-line blocks (that stalls PSUM
    drain -> stalls PE -> HAM cold).
  * Host: gather, transpose back, cast to f32.
"""

import os
import sys

sys.path.insert(0, "/opt/trn_rl_repo")

import numpy as np
import ml_dtypes

N_CORES = 8
B, S, D = 4, 4096, 2048
K = 4
PAD = 4
ROW = S + PAD  # 4100
D_LOCAL = D // N_CORES  # 256
G = D_LOCAL // 128  # 2 partition groups per core

MM_N = int(os.environ.get("KERNEL_MM_N", "512"))
IN_CHUNKS = int(os.environ.get("KERNEL_IN_CHUNKS", "2"))
# columns of the split tile computed on DVE (rest go to PE)
SPLIT_DVE_COLS = int(os.environ.get("KERNEL_SPLIT_DVE", "3584"))

_CACHE = {}

# ---- cost model (ns) for emission ordering -------------------------------
PE_START = 12000.0
DVE_START = 10700.0
PE_NS_PER_COL = 4.0 / 2.4  # 4 taps x 1 col / 2.4GHz


def _dve_chain_ns(w):
    ts = 4 * (w * 0.2604 + 210.0)
    tt = 3 * (w * 0.5208 + 150.0)
    return ts + tt


def _build():
    import concourse.tile as tile
    from concourse import bacc, mybir

    nc = bacc.Bacc("TRN2", debug=False, enable_asserts=False, num_devices=N_CORES)
    bf16 = mybir.dt.bfloat16
    f32 = mybir.dt.float32

    x_ap = nc.dram_tensor("x", [G, 128, B, ROW], bf16, kind="ExternalInput").ap()
    wd_ap = nc.dram_tensor("wd", [128, G * K * 128], bf16, kind="ExternalInput").ap()
    w_ap = nc.dram_tensor("w", [128, G * K], f32, kind="ExternalInput").ap()
    out_ap = nc.dram_tensor("out", [G, 128, B, S], bf16, kind="ExternalOutput").ap()

    DVE_TILES = (1, 3)
    SPLIT_TILE = 5

    with tile.TileContext(nc) as tc:
        with (
            tc.tile_pool(name="wp", bufs=1) as wp,
            tc.tile_pool(name="xp", bufs=8) as xp,
            tc.tile_pool(name="tp", bufs=2) as tp,
            tc.tile_pool(name="cp", bufs=2) as cp,
            tc.tile_pool(name="ps", bufs=2, space="PSUM") as ps,
            tc.tile_pool(name="yp", bufs=4) as yp,
        ):
            # weights via HWDGE on the scalar queue: fast setup, ACT is idle
            # until its first silu anyway
            wd = wp.tile([128, G * K * 128], bf16, tag="wd")
            wt = wp.tile([128, G * K], f32, tag="wt")
            nc.scalar.dma_start(out=wd[:], in_=wd_ap[:])
            nc.scalar.dma_start(out=wt[:], in_=w_ap[:])

            def wdiag(g, k):
                c0 = (g * K + k) * 128
                return wd[:, c0 : c0 + 128]

            # all input chunk DMAs first, in tile order, on the sync queue
            xts = []
            for ti in range(G * B):
                g, b = divmod(ti, B)
                xt = xp.tile([128, ROW], bf16, tag="xt")
                step = (ROW + IN_CHUNKS - 1) // IN_CHUNKS
                for c0 in range(0, ROW, step):
                    cw = min(step, ROW - c0)
                    nc.sync.dma_start(
                        out=xt[:, c0 : c0 + cw], in_=x_ap[g, :, b, c0 : c0 + cw]
                    )
                xts.append(xt)

            # ---- build unit worklist with modeled completion times -------
            units = []  # (ready_ns, kind, tile_idx, lo, hi)
            t_pe = PE_START
            t_dve = DVE_START
            pe_units = []
            for ti in range(G * B):
                if ti in DVE_TILES:
                    pe_units.append(None)
                elif ti == SPLIT_TILE:
                    pe_units.append([(SPLIT_DVE_COLS, S)])
                else:
                    pe_units.append([(0, 2048), (2048, S)])
            # PE chunk units in tile order
            for ti in range(G * B):
                if pe_units[ti] is None:
                    continue
                for lo, hi in pe_units[ti]:
                    t_pe += (hi - lo) * PE_NS_PER_COL
                    units.append((t_pe, "pe", ti, lo, hi))
            # DVE units: full tiles then the split tile (two half-chains)
            for ti in DVE_TILES:
                t_dve += _dve_chain_ns(S)
                units.append((t_dve, "dve", ti, 0, S))
            half = SPLIT_DVE_COLS // 2
            for lo, hi in ((0, half), (half, SPLIT_DVE_COLS)):
                t_dve += _dve_chain_ns(hi - lo)
                units.append((t_dve, "dve", SPLIT_TILE, lo, hi))

            units.sort(key=lambda u: u[0])

            def wcol(g, k):
                return wt[:, g * K + k : g * K + k + 1]

            def emit_pe(g, b, xt, lo, hi, last):
                cw = hi - lo
                y = yp.tile([128, cw], bf16, tag="y")
                acc = ps.tile([128, cw], f32, tag="acc")
                for k in range(K):
                    for n0 in range(0, cw, MM_N):
                        xlo = lo + n0 + 1 + k
                        nw = min(MM_N, cw - n0)
                        nc.tensor.matmul(
                            acc[:, n0 : n0 + nw],
                            wdiag(g, k),
                            xt[:, xlo : xlo + nw],
                            start=(k == 0),
                            stop=(k == K - 1),
                        )
                if last:
                    for s0 in range(0, cw, 1024):
                        sw = min(1024, cw - s0)
                        nc.scalar.activation(
                            out=y[:, s0 : s0 + sw],
                            in_=acc[:, s0 : s0 + sw],
                            func=mybir.ActivationFunctionType.Silu,
                        )
                    nc.scalar.dma_start(out=out_ap[g, :, b, lo:hi], in_=y[:])
                else:
                    nc.scalar.activation(
                        out=y[:], in_=acc[:], func=mybir.ActivationFunctionType.Silu
                    )
                    nc.gpsimd.dma_start(out=out_ap[g, :, b, lo:hi], in_=y[:])

            def emit_dve(g, b, xt, lo, hi, last):
                W = hi - lo
                ts = []
                for k in range(K):
                    t = tp.tile([128, W], bf16, tag=f"t{k % 2}")
                    nc.vector.tensor_scalar_mul(
                        t[:], xt[:, lo + 1 + k : lo + 1 + k + W], wcol(g, k)
                    )
                    ts.append(t)
                p0 = cp.tile([128, W], bf16, tag="p0")
                nc.vector.tensor_add(p0[:], ts[0][:], ts[1][:])
                p1 = cp.tile([128, W], bf16, tag="p1")
                nc.vector.tensor_add(p1[:], ts[2][:], ts[3][:])
                c = cp.tile([128, W], bf16, tag="c")
                nc.vector.tensor_add(c[:], p0[:], p1[:])
                y = yp.tile([128, W], bf16, tag="y")
                sw = 1024 if last else 2048
                for c0 in range(0, W, sw):
                    cw = min(sw, W - c0)
                    nc.scalar.activation(
                        out=y[:, c0 : c0 + cw],
                        in_=c[:, c0 : c0 + cw],
                        func=mybir.ActivationFunctionType.Silu,
                    )
                if last:
                    nc.scalar.dma_start(out=out_ap[g, :, b, lo:hi], in_=y[:])
                else:
                    for c0 in range(0, W, 2048):
                        cw = min(2048, W - c0)
                        nc.gpsimd.dma_start(
                            out=out_ap[g, :, b, lo + c0 : lo + c0 + cw],
                            in_=y[:, c0 : c0 + cw],
                        )

            for ui, (_, kind, ti, lo, hi) in enumerate(units):
                g, b = divmod(ti, B)
                last = ui == len(units) - 1
                if kind == "pe":
                    emit_pe(g, b, xts[ti], lo, hi, last)
                else:
                    emit_dve(g, b, xts[ti], lo, hi, last)

    nc.compile()
    return nc


def _get_nc():
    if "nc" not in _CACHE:
        _CACHE["nc"] = _build()
    return _CACHE["nc"]


def _make_in_maps(x, w):
    x = np.asarray(x, dtype=np.float32)
    w = np.asarray(w, dtype=np.float32)

    # (B, S, D) -> (D, B, S), bf16, left-pad rows with PAD zeros.
    x_t = np.ascontiguousarray(x.transpose(2, 0, 1)).astype(ml_dtypes.bfloat16)
    x_pad = np.zeros((D, B, ROW), dtype=ml_dtypes.bfloat16)
    x_pad[:, :, PAD:] = x_t
    w_flat = np.ascontiguousarray(w[:, 0, :])  # (D, K) f32

    in_maps = []
    for i in range(N_CORES):
        lo, hi = i * D_LOCAL, (i + 1) * D_LOCAL
        m = {"x": np.ascontiguousarray(x_pad[lo:hi].reshape(G, 128, B, ROW))}
        m["w"] = np.ascontiguousarray(
            w_flat[lo:hi].reshape(G, 128, K).transpose(1, 0, 2).reshape(128, G * K)
        )
        # diag stationaries, laid out [128, G*K*128] partition-first
        wd = np.zeros((G, K, 128, 128), dtype=ml_dtypes.bfloat16)
        wl = w_flat[lo:hi].reshape(G, 128, K).astype(ml_dtypes.bfloat16)
        idx = np.arange(128)
        for g in range(G):
            for k in range(K):
                wd[g, k, idx, idx] = wl[g, :, k]
        # (G,K,p,m) -> (p, G,K,m) -> [128, G*K*128]
        m["wd"] = np.ascontiguousarray(
            wd.transpose(2, 0, 1, 3).reshape(128, G * K * 128)
        )
        in_maps.append(m)
    return in_maps


def _assemble(results):
    parts = []
    for r in results:
        y = np.asarray(r["out"]).reshape(D_LOCAL, B, S)
        parts.append(y)
    y_full = np.concatenate(parts, axis=0)  # (D, B, S) bf16
    return np.ascontiguousarray(y_full.transpose(1, 2, 0)).astype(np.float32)


def kernel(x, w):
    from concourse.bass_utils import run_bass_kernel_spmd

    nc = _get_nc()
    in_maps = _make_in_maps(x, w)
    trace = bool(int(os.environ.get("KERNEL_TRACE", "0")))
    res = None
    err = None
    for attempt in range(3):
        try:
            res = run_bass_kernel_spmd(
                nc, in_maps, core_ids=list(range(N_CORES)),
                trace=trace and attempt == 0,
            )
            break
        except Exception as e:  # transient NRT device errors / missing trace hook
            err = e
            os.environ["BASS_NEVER_TRACE"] = "1"
            trace = False
    if res is None:
        raise err
    _CACHE["last_results"] = res
    return _assemble(res.results)


# revision 12
# speedup vs baseline: 1.2972x; 1.0138x over previous
"""Trainium2 Bass kernel: causal depthwise Conv1d (K=4) + SiLU.

Reference computation (B=4, S=4096, D=2048):
    y[b, s, d] = silu( sum_k w[d, 0, k] * x[b, s-3+k, d] )   (zero-padded left)

Strategy:
  * Host: transpose x to channel-major (D, B, S), left-pad each row with
    4 zeros (row length 4100), cast to bf16, shard D across the 8
    NeuronCores (256 channels each).  Depthwise conv is channel-independent
    -> no inter-core communication.
  * Core: 8 tiles of [128, 4100].  Tiles are computed on the TensorEngine
    (diag-stationary matmuls accumulate the 4 taps in PSUM; ACT silu drains
    PSUM -> bf16) or the VectorEngine (4 tensor_scalar muls @4x + 3 adds
    @2x; ACT silu).  One tile is split between the two for balance.
    Compute/drain units are emitted in modeled completion order so the
    strict-FIFO ACT queue never head-of-line blocks (that stalls PSUM
    drain -> stalls PE -> HAM cold).
  * Host: gather, transpose back, cast to f32.
"""

import os
import sys

sys.path.insert(0, "/opt/trn_rl_repo")

import numpy as np
import ml_dtypes

N_CORES = 8
B, S, D = 4, 4096, 2048
K = 4
PAD = 4
ROW = S + PAD  # 4100
D_LOCAL = D // N_CORES  # 256
G = D_LOCAL // 128  # 2 partition groups per core

MM_N = int(os.environ.get("KERNEL_MM_N", "512"))
IN_CHUNKS = int(os.environ.get("KERNEL_IN_CHUNKS", "2"))
# columns of the split tile computed on DVE (rest go to PE)
SPLIT_DVE_COLS = int(os.environ.get("KERNEL_SPLIT_DVE", "2560"))
WARMUP_MMS = int(os.environ.get("KERNEL_WARMUP", "9"))

_CACHE = {}

# ---- cost model (ns) for emission ordering -------------------------------
PE_START = 12000.0
DVE_START = 10700.0
PE_NS_PER_COL = 4.0 / 2.4  # 4 taps x 1 col / 2.4GHz


def _dve_chain_ns(w):
    ts = 4 * (w * 0.2604 + 210.0)
    tt = 3 * (w * 0.5208 + 150.0)
    return ts + tt


def _build():
    import concourse.tile as tile
    from concourse import bacc, mybir

    nc = bacc.Bacc("TRN2", debug=False, enable_asserts=False, num_devices=N_CORES)
    bf16 = mybir.dt.bfloat16
    f32 = mybir.dt.float32

    x_ap = nc.dram_tensor("x", [G, 128, B, ROW], bf16, kind="ExternalInput").ap()
    wd_ap = nc.dram_tensor("wd", [128, G * K * 128], bf16, kind="ExternalInput").ap()
    w_ap = nc.dram_tensor("w", [128, G * K], f32, kind="ExternalInput").ap()
    out_ap = nc.dram_tensor("out", [G, 128, B, S], bf16, kind="ExternalOutput").ap()

    DVE_TILES = (1, 3)
    SPLIT_TILE = 5

    with tile.TileContext(nc) as tc:
        with (
            tc.tile_pool(name="wp", bufs=1) as wp,
            tc.tile_pool(name="xp", bufs=8) as xp,
            tc.tile_pool(name="tp", bufs=2) as tp,
            tc.tile_pool(name="cp", bufs=2) as cp,
            tc.tile_pool(name="ps", bufs=2, space="PSUM") as ps,
            tc.tile_pool(name="yp", bufs=4) as yp,
        ):
            # weights first on the sync queue (HWDGE, fast) so wd gates the
            # PE warmup as early as possible; issuing them on scalar causes a
            # second ACT_TABLE_LOAD (walrus invalidates table tracking)
            wd = wp.tile([128, G * K * 128], bf16, tag="wd")
            wt = wp.tile([128, G * K], f32, tag="wt")
            nc.sync.dma_start(out=wd[:], in_=wd_ap[:])
            nc.sync.dma_start(out=wt[:], in_=w_ap[:])

            def wdiag(g, k):
                c0 = (g * K + k) * 128
                return wd[:, c0 : c0 + 128]

            # all input chunk DMAs next, in tile order, on the sync queue
            xts = []
            for ti in range(G * B):
                g, b = divmod(ti, B)
                xt = xp.tile([128, ROW], bf16, tag="xt")
                # chunk boundary at 2052 so a 2048-col compute half (which
                # reads up to col lo+2051) depends on only one chunk
                bounds = [0, 2052, ROW] if IN_CHUNKS == 2 else [0, ROW]
                for c0, c1 in zip(bounds, bounds[1:]):
                    nc.sync.dma_start(
                        out=xt[:, c0:c1], in_=x_ap[g, :, b, c0:c1]
                    )
                xts.append(xt)

            # HAM warmup: dummy matmuls as soon as wd lands keep the PE busy
            # through the ~3.4us activity window, so the first real chunk
            # runs at 2.4GHz instead of 1.2GHz. Result is never read.
            if WARMUP_MMS:
                # same tag/shape as real chunks: rotates through the acc
                # buffers, no extra PSUM footprint (it has no readers)
                warm = ps.tile([128, 2048], f32, tag="acc")
                for _ in range(WARMUP_MMS):
                    nc.tensor.matmul(
                        warm[:, 0:MM_N], wdiag(0, 0), wd[:, 0:MM_N],
                        start=True, stop=True,
                    )

            # ---- build unit worklist with modeled completion times -------
            # kinds: "pe" (chunk: matmuls+silu+dma), "dvec" (vector chain),
            # "dves" (silu+dma for a sub-range of a finished chain)
            units = []  # (ready_ns, kind, tile_idx, lo, hi)
            t_pe = PE_START
            t_dve = DVE_START
            pe_units = []
            for ti in range(G * B):
                if ti in DVE_TILES:
                    pe_units.append(None)
                elif ti == SPLIT_TILE:
                    pe_units.append([(SPLIT_DVE_COLS, S)])
                else:
                    pe_units.append([(0, 2048), (2048, S)])
            for ti in range(G * B):
                if pe_units[ti] is None:
                    continue
                for lo, hi in pe_units[ti]:
                    t_pe += (hi - lo) * PE_NS_PER_COL
                    units.append((t_pe, "pe", ti, lo, hi))

            def add_dve_chain(ti, lo, hi):
                nonlocal t_dve
                t_dve += _dve_chain_ns(hi - lo)
                units.append((t_dve, "dvec", ti, lo, hi))
                # silu/dma drains in 2048 sub-chunks, interleaved with PE
                # units by the sort so ACT never head-of-line blocks PSUM
                for i, c0 in enumerate(range(lo, hi, 2048)):
                    cw = min(2048, hi - c0)
                    units.append((t_dve + 1 + 2500 * i, "dves", ti, c0, c0 + cw))

            d0 = DVE_TILES[0]
            add_dve_chain(d0, 0, 2048)
            add_dve_chain(d0, 2048, S)
            for ti in DVE_TILES[1:]:
                add_dve_chain(ti, 0, S)
            half = SPLIT_DVE_COLS // 2
            add_dve_chain(SPLIT_TILE, 0, half)
            add_dve_chain(SPLIT_TILE, half, SPLIT_DVE_COLS)

            units.sort(key=lambda u: u[0])

            def wcol(g, k):
                return wt[:, g * K + k : g * K + k + 1]

            def emit_pe(g, b, xt, lo, hi, last):
                cw = hi - lo
                y = yp.tile([128, cw], bf16, tag="y")
                acc = ps.tile([128, cw], f32, tag="acc")
                for k in range(K):
                    for n0 in range(0, cw, MM_N):
                        xlo = lo + n0 + 1 + k
                        nw = min(MM_N, cw - n0)
                        nc.tensor.matmul(
                            acc[:, n0 : n0 + nw],
                            wdiag(g, k),
                            xt[:, xlo : xlo + nw],
                            start=(k == 0),
                            stop=(k == K - 1),
                        )
                if last:
                    for s0 in range(0, cw, 1024):
                        sw = min(1024, cw - s0)
                        nc.scalar.activation(
                            out=y[:, s0 : s0 + sw],
                            in_=acc[:, s0 : s0 + sw],
                            func=mybir.ActivationFunctionType.Silu,
                        )
                    nc.scalar.dma_start(out=out_ap[g, :, b, lo:hi], in_=y[:])
                else:
                    nc.scalar.activation(
                        out=y[:], in_=acc[:], func=mybir.ActivationFunctionType.Silu
                    )
                    nc.gpsimd.dma_start(out=out_ap[g, :, b, lo:hi], in_=y[:])

            cbufs = {}  # (ti, 2048-chunk lo) -> (c tile, chain lo)

            def emit_dve_chain(g, b, ti, xt, lo, hi):
                W = hi - lo
                ts = []
                for k in range(K):
                    t = tp.tile([128, W], bf16, tag=f"t{k % 2}")
                    nc.vector.tensor_scalar_mul(
                        t[:], xt[:, lo + 1 + k : lo + 1 + k + W], wcol(g, k)
                    )
                    ts.append(t)
                p0 = cp.tile([128, W], bf16, tag="p0")
                nc.vector.tensor_add(p0[:], ts[0][:], ts[1][:])
                p1 = cp.tile([128, W], bf16, tag="p1")
                nc.vector.tensor_add(p1[:], ts[2][:], ts[3][:])
                c = cp.tile([128, W], bf16, tag="c")
                nc.vector.tensor_add(c[:], p0[:], p1[:])
                for c0 in range(lo, hi, 2048):
                    cbufs[(ti, c0)] = (c, lo)

            def emit_dve_silu(g, b, ti, lo, hi, last):
                c, chain_lo = cbufs[(ti, lo)]
                W = hi - lo
                y = yp.tile([128, W], bf16, tag="y")
                sw = 1024 if last else 2048
                for c0 in range(0, W, sw):
                    cw = min(sw, W - c0)
                    nc.scalar.activation(
                        out=y[:, c0 : c0 + cw],
                        in_=c[:, lo - chain_lo + c0 : lo - chain_lo + c0 + cw],
                        func=mybir.ActivationFunctionType.Silu,
                    )
                if last:
                    nc.scalar.dma_start(out=out_ap[g, :, b, lo:hi], in_=y[:])
                else:
                    nc.gpsimd.dma_start(out=out_ap[g, :, b, lo:hi], in_=y[:])

            for ui, (_, kind, ti, lo, hi) in enumerate(units):
                g, b = divmod(ti, B)
                last = ui == len(units) - 1
                if kind == "pe":
                    emit_pe(g, b, xts[ti], lo, hi, last)
                elif kind == "dvec":
                    emit_dve_chain(g, b, ti, xts[ti], lo, hi)
                else:
                    emit_dve_silu(g, b, ti, lo, hi, last)

    nc.compile()
    return nc


def _get_nc():
    if "nc" not in _CACHE:
        _CACHE["nc"] = _build()
    return _CACHE["nc"]


def _make_in_maps(x, w):
    x = np.asarray(x, dtype=np.float32)
    w = np.asarray(w, dtype=np.float32)

    # (B, S, D) -> (D, B, S), bf16, left-pad rows with PAD zeros.
    x_t = np.ascontiguousarray(x.transpose(2, 0, 1)).astype(ml_dtypes.bfloat16)
    x_pad = np.zeros((D, B, ROW), dtype=ml_dtypes.bfloat16)
    x_pad[:, :, PAD:] = x_t
    w_flat = np.ascontiguousarray(w[:, 0, :])  # (D, K) f32

    in_maps = []
    for i in range(N_CORES):
        lo, hi = i * D_LOCAL, (i + 1) * D_LOCAL
        m = {"x": np.ascontiguousarray(x_pad[lo:hi].reshape(G, 128, B, ROW))}
        m["w"] = np.ascontiguousarray(
            w_flat[lo:hi].reshape(G, 128, K).transpose(1, 0, 2).reshape(128, G * K)
        )
        # diag stationaries, laid out [128, G*K*128] partition-first
        wd = np.zeros((G, K, 128, 128), dtype=ml_dtypes.bfloat16)
        wl = w_flat[lo:hi].reshape(G, 128, K).astype(ml_dtypes.bfloat16)
        idx = np.arange(128)
        for g in range(G):
            for k in range(K):
                wd[g, k, idx, idx] = wl[g, :, k]
        # (G,K,p,m) -> (p, G,K,m) -> [128, G*K*128]
        m["wd"] = np.ascontiguousarray(
            wd.transpose(2, 0, 1, 3).reshape(128, G * K * 128)
        )
        in_maps.append(m)
    return in_maps


def _assemble(results):
    parts = []
    for r in results:
        y = np.asarray(r["out"]).reshape(D_LOCAL, B, S)
        parts.append(y)
    y_full = np.concatenate(parts, axis=0)  # (D, B, S) bf16
    return np.ascontiguousarray(y_full.transpose(1, 2, 0)).astype(np.float32)


def kernel(x, w):
    from concourse.bass_utils import run_bass_kernel_spmd

    nc = _get_nc()
    in_maps = _make_in_maps(x, w)
    trace = bool(int(os.environ.get("KERNEL_TRACE", "0")))
    res = None
    err = None
    for attempt in range(3):
        try:
            res = run_bass_kernel_spmd(
                nc, in_maps, core_ids=list(range(N_CORES)),
                trace=trace and attempt == 0,
            )
            break
        except Exception as e:  # transient NRT device errors / missing trace hook
            err = e
            os.environ["BASS_NEVER_TRACE"] = "1"
            trace = False
    if res is None:
        raise err
    _CACHE["last_results"] = res
    return _assemble(res.results)
